# revision 5
# baseline (speedup 1.0000x reference)
"""Binarized linear kernel for Trainium2 (8 NeuronCores).

Problem: per-direction binary "match count" GEMM.
  input        (B=64, D=128, I=512)  bool
  weight_noise (D=128, O=512, I=512) bool
  bias_noise   (D=128, O=512)        float32
  out[b,d,o] = (#matches(input[b,d,:], weight_noise[d,:,:]) > bias_noise[d,o])

Math: with +/-1 encoding x~=2x-1, w~=2w-1:
  matches = (I + sum_i x~ w~) / 2, so
  out = (dotpm > 2*bias - I), where dotpm is a single +/-1 GEMM per direction.
Host pre-encodes +/-1 in fp8_e4m3 (exact), precomputes thr = 2*bias - I (exact
in fp32: 2*bias is exact; subtracting 512 from values in [256,1024] is exact by
Sterbenz). PSUM accumulates fp32 exactly (integers <= 512), so the comparison
is bit-identical to the reference.

Sharding: D across the 8 cores (16 directions each), fully independent.
"""

import numpy as np

import sys

for _p in ("/opt/trn_rl_repo",):
    if _p not in sys.path:
        sys.path.insert(0, _p)

B, D, O, I = 64, 128, 512, 512
NCORES = 8
DL = D // NCORES  # directions per core
KC = I // 128     # contraction chunks of 128

_NC_CACHE = {}


def _build_bass():
    import concourse.mybir as mybir
    from concourse import bacc
    from concourse.tile import TileContext

    fp8 = mybir.dt.float8e4
    f32 = mybir.dt.float32
    u8 = mybir.dt.uint8

    nc = bacc.Bacc("TRN2")
    xt_d = nc.dram_tensor("xt", [128, DL * KC * B], fp8, kind="ExternalInput")
    wt_d = nc.dram_tensor("wt", [DL, 128, KC * O], fp8, kind="ExternalInput")
    thr_d = nc.dram_tensor("thr", [B, DL * O], f32, kind="ExternalInput")
    out_d = nc.dram_tensor("out", [B, DL * O], u8, kind="ExternalOutput")

    with TileContext(nc) as tc:
        with (
            tc.tile_pool(name="cpool", bufs=1) as cpool,
            tc.tile_pool(name="wpool", bufs=DL) as wpool,
            tc.tile_pool(name="ppool", bufs=4, space="PSUM") as ppool,
        ):
            x_tile = cpool.tile([128, DL * KC * B], fp8)
            nc.sync.dma_start(out=x_tile[:], in_=xt_d[:])
            thr_t = cpool.tile([B, DL * O], f32)
            nc.sync.dma_start(out=thr_t[:], in_=thr_d[:])
            out_sb = cpool.tile([B, DL * O], u8)
            # Absorb the thr DMA wait on DVE here: the per-direction
            # TensorTensor below can then carry only the PE wait (the
            # S3S3D3_TT struct has a single sync-wait slot).
            scratch = cpool.tile([1, 8], f32)
            nc.vector.tensor_copy(out=scratch[:1, :8], in_=thr_t[:1, :8])

            xv = x_tile.rearrange("k (d c b) -> k d c b", d=DL, c=KC)
            for d in range(DL):
                w_tile = wpool.tile([128, KC * O], fp8)
                nc.sync.dma_start(out=w_tile[:], in_=wt_d[d, :, :])
                wv = w_tile.rearrange("k (c o) -> k c o", c=KC)
                psum = ppool.tile([B, O], f32)
                for c in range(KC):
                    nc.tensor.matmul(
                        psum[:],
                        xv[:, d, c, :],
                        wv[:, c, :],
                        start=(c == 0),
                        stop=(c == KC - 1),
                    )
                nc.vector.tensor_tensor(
                    out=out_sb[:, d * O : (d + 1) * O],
                    in0=psum[:],
                    in1=thr_t[:, d * O : (d + 1) * O],
                    op=mybir.AluOpType.is_gt,
                )
            nc.sync.dma_start(out=out_d[:], in_=out_sb[:])
    nc.compile()
    return nc


def _get_nc():
    if "nc" not in _NC_CACHE:
        _NC_CACHE["nc"] = _build_bass()
    return _NC_CACHE["nc"]


def _prep_inputs(input, weight_noise, bias_noise):
    import ml_dtypes

    fp8 = ml_dtypes.float8_e4m3
    x = np.asarray(input).astype(np.int8)  # (B, D, I) in {0,1}
    w = np.asarray(weight_noise).astype(np.int8)  # (D, O, I)
    bias = np.asarray(bias_noise).astype(np.float32)  # (D, O)

    xs = (2 * x - 1).astype(fp8)  # +/-1
    ws = (2 * w - 1).astype(fp8)
    thr = (np.float32(2.0) * bias - np.float32(I)).astype(np.float32)

    in_maps = []
    for c in range(NCORES):
        dsl = slice(c * DL, (c + 1) * DL)
        # xt[k, d, cc, b] = xs[b, d0+d, cc*128+k]
        xt = xs[:, dsl, :].transpose(2, 1, 0)  # (I, DL, B)
        xt = xt.reshape(KC, 128, DL, B).transpose(1, 2, 0, 3)  # (k, d, cc, b)
        xt = np.ascontiguousarray(xt).reshape(128, DL * KC * B)
        # wt[d, k, cc, o] = ws[d0+d, o, cc*128+k]
        wt = ws[dsl].transpose(0, 2, 1)  # (DL, I, O)
        wt = wt.reshape(DL, KC, 128, O).transpose(0, 2, 1, 3)  # (d, k, cc, o)
        wt = np.ascontiguousarray(wt).reshape(DL, 128, KC * O)
        th = np.ascontiguousarray(
            np.broadcast_to(thr[dsl].reshape(1, DL * O), (B, DL * O))
        )
        in_maps.append({"xt": xt, "wt": wt, "thr": th})
    return in_maps


def kernel(input, weight_noise, bias_noise):
    from concourse import bass_utils

    in_maps = _prep_inputs(input, weight_noise, bias_noise)
    nc = _get_nc()
    res = bass_utils.run_bass_kernel_spmd(nc, in_maps, core_ids=list(range(NCORES)))
    outs = [np.asarray(r["out"]).reshape(B, DL, O) for r in res.results]
    full = np.concatenate(outs, axis=1)  # (B, D, O)
    return full.astype(bool)
